# revision 2
# baseline (speedup 1.0000x reference)
"""Trainium2 Bass kernel for nn_Deepset (GNN message passing).

Computation:
    h = relu(x @ W1 + b1)                   # [400000, 1024] @ [1024, 64]
    pooled = segment_mean(h, batch, 512)    # batch is sorted
    out = softmax(head(pooled), axis=0)     # tiny MLP head, on host

Segment-mean commutes with the affine W2 layer, so the device only needs
per-graph sums of h; everything downstream of the [512, 64] sums runs on
host in float64.

Device design (8 cores, data-parallel over nodes, 50000 nodes/core):
  - x is scaled by 16 and cast to fp8 e4m3 on host: 51.4 MB/core of HBM
    traffic (half of bf16), within the 2e-2 output tolerance (measured
    rel err ~5.6e-3).
  - Main GEMM: fp8 DoubleRow matmuls (virtual K=256, 4 matmuls per
    512-node tile) -> ~2x PE throughput vs bf16 at M=64. Weights are
    scaled by 16 too; the relu activation applies scale 1/256.
  - Two 512-node tiles share one 1 MB DMA (pair-contiguous layout) to
    stay on the DMA bandwidth curve.
  - Segment-sum tail runs off the PE: the host repacks nodes so every
    64-node sub-chunk belongs to a single graph (each graph's run padded
    to a multiple of 64 with zero rows); ScalarE does relu+bias+cast,
    VectorE does a per-sub-chunk tensor_reduce, and the host assembles
    per-graph sums from the [64, n_chunks] partials.
"""

import numpy as np

N_NODES = 400000
D_FEAT = 1024
HIDDEN = 64
NUM_GRAPHS = 512
N_CORES = 8
NPC = N_NODES // N_CORES        # 50000 nodes per core
TILE_N = 512                    # nodes per PE tile
SUB = 64                        # single-graph sub-chunk granularity
N_PAD = 52224                   # 102 * 512 padded nodes per core
N_TILES = N_PAD // TILE_N       # 102
SPT = TILE_N // SUB             # sub-chunks per tile (8)
KC = D_FEAT // 128              # 8 k-chunks
X_SCALE = 16.0                  # x, W1 scaled by 16 before fp8 cast
ACT_SCALE = 1.0 / (X_SCALE * X_SCALE)

LAST_RESULT = None


def _build_nc(repeat=1, group=3, xp_bufs=16, ps_bufs=8):
    import concourse.bass as bass
    import concourse.bacc as bacc
    import concourse.tile as tile
    from concourse import mybir
    from contextlib import ExitStack

    dt = mybir.dt
    fp8 = dt.float8e4
    T = N_TILES

    nc = bacc.Bacc("TRN2", target_bir_lowering=False, debug=False)
    xT = nc.declare_dram_parameter("xT", [T // 2, 128, 2 * KC * TILE_N],
                                   fp8, isOutput=False)
    w1 = nc.declare_dram_parameter("w1", [128, KC, HIDDEN], fp8, isOutput=False)
    b1 = nc.declare_dram_parameter("b1", [HIDDEN, 1], dt.float32,
                                   isOutput=False)
    sout = nc.declare_dram_parameter("sout", [HIDDEN, T * SPT], dt.float32,
                                     isOutput=True)

    with ExitStack() as ctx:
        tc = ctx.enter_context(tile.TileContext(nc))
        const = ctx.enter_context(tc.tile_pool(name="const", bufs=1))
        xp = ctx.enter_context(tc.tile_pool(name="xp", bufs=xp_bufs))
        hp = ctx.enter_context(tc.tile_pool(name="hp", bufs=ps_bufs,
                                            space=bass.MemorySpace.PSUM))
        hs = ctx.enter_context(tc.tile_pool(name="hs", bufs=4))

        w1_sb = const.tile([128, KC, HIDDEN], fp8, name="w1_sb")
        nc.sync.dma_start(w1_sb[:], w1[:, :, :])
        b1_sb = const.tile([HIDDEN, 1], dt.float32, name="b1_sb")
        nc.sync.dma_start(b1_sb[:], b1[:, :])
        sums_sb = const.tile([HIDDEN, T * SPT], dt.float32, name="sums_sb")

        from concourse.mybir import (MatmulPerfMode, ActivationFunctionType,
                                     AluOpType, AxisListType)

        for r in range(repeat):
            for g0 in range(0, T // 2, group):
                g1 = min(g0 + group, T // 2)
                xts = []
                for q in range(g0, g1):
                    xt = xp.tile([128, 2, KC // 2, 2, TILE_N], fp8,
                                 tag="xt", name="xt")
                    nc.sync.dma_start(
                        xt[:],
                        xT[q, :, :].rearrange("p (u a b n) -> p u a b n",
                                              u=2, a=KC // 2, b=2))
                    xts.append(xt)
                hts = [[hp.tile([HIDDEN, TILE_N], dt.float32, tag="ht",
                                name="ht") for _ in range(2)]
                       for _ in range(g0, g1)]
                for kp in range(KC // 2):
                    for i, q in enumerate(range(g0, g1)):
                        for u in range(2):
                            nc.tensor.matmul(
                                hts[i][u][:],
                                w1_sb[:, 2 * kp:2 * kp + 2, :],
                                xts[i][:, u, kp],
                                start=(kp == 0), stop=(kp == KC // 2 - 1),
                                perf_mode=MatmulPerfMode.DoubleRow)
                for i, q in enumerate(range(g0, g1)):
                    for u in range(2):
                        t = 2 * q + u
                        h_sb = hs.tile([HIDDEN, TILE_N], dt.bfloat16,
                                       tag="hsb", name="hsb")
                        nc.scalar.activation(h_sb[:], hts[i][u][:],
                                             ActivationFunctionType.Relu,
                                             bias=b1_sb[:], scale=ACT_SCALE)
                        nc.vector.tensor_reduce(
                            sums_sb[:, t * SPT:(t + 1) * SPT],
                            h_sb[:].rearrange("p (c s) -> p c s", c=SPT),
                            axis=AxisListType.X, op=AluOpType.add)

        nc.sync.dma_start(sout[:, :], sums_sb[:])

    nc.compile()
    return nc


def _repack(x, batch):
    """Per-core: pad each graph's node run to a multiple of SUB, build the
    per-sub-chunk graph map, pack x into the fp8 DoubleRow tile layout."""
    import ml_dtypes
    fp8 = np.dtype(ml_dtypes.float8_e4m3)

    batch = np.asarray(batch, np.int64)
    x = np.asarray(x, np.float32)

    in_maps = []
    chunk_graphs = []
    pad_per_graph = np.zeros(NUM_GRAPHS, np.int64)
    for c in range(N_CORES):
        lo, hi = c * NPC, (c + 1) * NPC
        b = batch[lo:hi]
        starts = np.concatenate([[0], np.flatnonzero(np.diff(b)) + 1])
        ends = np.concatenate([starts[1:], [NPC]])
        gids = b[starts]
        gidx = np.full(N_PAD, -1, np.int64)
        cg = np.full(N_PAD // SUB, -1, np.int64)
        pos = 0
        for s, e, g in zip(starts, ends, gids):
            cnt = e - s
            padded = -(-cnt // SUB) * SUB
            if pos + padded > N_PAD:
                raise OverflowError("N_PAD too small for this batch")
            gidx[pos:pos + cnt] = lo + s + np.arange(cnt)
            cg[pos // SUB:(pos + padded) // SUB] = g
            pad_per_graph[g] += padded - cnt
            pos += padded
        chunk_graphs.append(cg)

        xs = np.zeros((N_PAD, D_FEAT), dtype=fp8)
        valid = gidx >= 0
        xs[valid] = np.clip(x[gidx[valid]] * X_SCALE, -240, 240).astype(fp8)
        # feat = kp*256 + j*128 + p  ->  [T, 128, kp, j, n]
        xT = np.ascontiguousarray(
            xs.reshape(N_TILES, TILE_N, KC // 2, 2, 128)
            .transpose(0, 4, 2, 3, 1)
        ).reshape(N_TILES, 128, KC * TILE_N)
        # pair-contiguous per partition: [T//2, 128, 2*4096] -> 1 MB DMAs
        xT = np.ascontiguousarray(
            xT.reshape(N_TILES // 2, 2, 128, KC * TILE_N)
            .transpose(0, 2, 1, 3)
        ).reshape(N_TILES // 2, 128, 2 * KC * TILE_N)
        in_maps.append({"xT": xT})
    return in_maps, chunk_graphs, pad_per_graph


def _prep_weights(W1, b1):
    import ml_dtypes
    fp8 = np.dtype(ml_dtypes.float8_e4m3)
    W1 = np.asarray(W1, np.float32)
    b1 = np.asarray(b1, np.float32)
    # feat = idx*128 + p with idx = kp*2 + j -> [128, idx, h]
    w1p = np.ascontiguousarray(
        np.clip(W1 * X_SCALE, -240, 240)
        .reshape(KC, 128, HIDDEN).transpose(1, 0, 2)).astype(fp8)
    b1p = b1.reshape(HIDDEN, 1).copy()
    return w1p, b1p


def _assemble(results, chunk_graphs, pad_per_graph, b1):
    """Per-core device sub-chunk sums -> per-graph sums S [512, 64] f64."""
    S = np.zeros((NUM_GRAPHS, HIDDEN), np.float64)
    for c in range(N_CORES):
        sums = np.asarray(results[c]["sout"], np.float64)  # [64, T*SPT]
        cg = chunk_graphs[c]
        valid = cg >= 0
        np.add.at(S, cg[valid], sums[:, valid].T)
    # padded (zero) rows each contribute relu(b1)
    relu_b1 = np.maximum(np.asarray(b1, np.float64), 0.0)
    S -= pad_per_graph[:, None] * relu_b1[None, :]
    return S


def _head(S, batch, W2, b2, W3, b3, W4, b4):
    cnt = np.bincount(np.asarray(batch, np.int64),
                      minlength=NUM_GRAPHS).astype(np.float64)
    meanh = S / np.maximum(cnt, 1.0)[:, None]
    pooled = meanh @ np.asarray(W2, np.float64) + np.asarray(b2, np.float64)
    pooled *= (cnt > 0)[:, None]  # empty graphs pool to exactly zero
    z = pooled @ np.asarray(W3, np.float64) + np.asarray(b3, np.float64)
    z = z @ np.asarray(W4, np.float64) + np.asarray(b4, np.float64)
    z -= z.max(axis=0, keepdims=True)
    e = np.exp(z)
    return (e / e.sum(axis=0, keepdims=True)).astype(np.float32)


def kernel(x, batch, W1, b1, W2, b2, W3, b3, W4, b4):
    global LAST_RESULT
    from concourse.bass_utils import run_bass_kernel_spmd

    x = np.asarray(x)
    batch = np.asarray(batch)
    in_maps, chunk_graphs, pad_per_graph = _repack(x, batch)
    w1p, b1p = _prep_weights(W1, b1)
    for m in in_maps:
        m["w1"] = w1p
        m["b1"] = b1p

    nc = _build_nc()
    res = run_bass_kernel_spmd(nc, in_maps, list(range(N_CORES)))
    LAST_RESULT = res

    S = _assemble(res.results, chunk_graphs, pad_per_graph, b1)
    return _head(S, batch, W2, b2, W3, b3, W4, b4)
